# revision 25
# baseline (speedup 1.0000x reference)
"""Trainium2 Bass kernel for HardQuadRadiusTripletLoss.

Device computes, per image (one per NeuronCore), a PE-folded dense
correlation top-8. Cells are paired (j, j+1800); for each pair the PE
computes M = b + relu(a-b) = max(a, b) directly in PSUM:
  - mm_b:   sims of the b-side cells   (fp8e4m3 DoubleRow, K=256, 0.5 cy/row)
  - mm_d:   D = a-b sims from a host-precomputed difference map (fp8 DR)
  - ACT:    u = relu(D) -> bf16 SBUF   (the only per-element ACT pass)
  - acc:    M += Identity @ u          (bf16 matmul accumulate into mm_b PSUM)
  - DVE:    max8 over 900 pairs (strided 2-bank PSUM read) -> f16 top8
This halves the DVE scan (1800 pairs instead of 3600 cells per keypoint
tile) and eliminates the mask pipeline entirely.

Host does the cheap/small work: fp8 packing, the positive-cell similarity
(exact fp32), and the grid-radius mask: for each of the <=9 candidate masked
cells it recomputes the pair's (b, D) through the same quantized arithmetic,
removes the pair-max from the device top-16 when the masked side won the
pair, and inserts the surviving partner value. Squared-hinge loss in fp64.

Sharding: data-parallel over batch B=8 -> one image per core.
"""

import sys

if "/opt/trn_rl_repo" not in sys.path:
    sys.path.insert(0, "/opt/trn_rl_repo")

import numpy as np
import ml_dtypes

B, N, C, H, W = 8, 2048, 256, 60, 60
HW = H * W            # 3600
P = HW // 2           # 1800 pairs
GRID = 8.0
NTILE = N // 128      # 16
NH = 2 * NTILE        # 32 half-tiles (900 pairs each)
SCALE = np.float32(16.0)
SCALE2 = np.float32(SCALE * SCALE)

F8 = ml_dtypes.float8_e4m3  # matches mybir.dt.float8e4
BF16 = ml_dtypes.bfloat16

_NC_CACHE = {}
_HOST_CTX = {}


def _build_nc():
    from concourse import bacc, mybir
    import concourse.tile as tile

    nc = bacc.Bacc("TRN2", target_bir_lowering=False, debug=False)
    f32 = mybir.dt.float32
    f16 = mybir.dt.float16
    bf16 = mybir.dt.bfloat16
    fp8 = mybir.dt.float8e4
    Act = mybir.ActivationFunctionType
    DR = mybir.MatmulPerfMode.DoubleRow

    d_lhsT = nc.dram_tensor("lhsT", (128, 4, 2, 512), fp8, kind="ExternalInput").ap()
    d_rhsb = nc.dram_tensor("rhsb", (128, 4, 2, 450), fp8, kind="ExternalInput").ap()
    d_rhsd = nc.dram_tensor("rhsd", (128, 4, 2, 450), fp8, kind="ExternalInput").ap()
    d_ident = nc.dram_tensor("ident", (128, 128), bf16, kind="ExternalInput").ap()
    d_top8 = nc.dram_tensor("top8", (N, 16), f16, kind="ExternalOutput").ap()

    with tile.TileContext(nc) as tc:
        with (
            tc.tile_pool(name="pers", bufs=1) as pers,
            tc.tile_pool(name="ub", bufs=4) as ubp,
            tc.tile_pool(name="outp", bufs=6) as outp,
            tc.tile_pool(name="psm", bufs=2, space="PSUM") as psm,
            tc.tile_pool(name="psd", bufs=2, space="PSUM") as psd,
        ):
            # contiguous input DMAs ordered so step 0's operands land first
            # (lhsT quarter 0, then the low rhs halves); DMA_ENGINES is a
            # serialized resource in the cost model, so order = latency
            ident = pers.tile([128, 128], bf16, tag="ident")
            rhsd_lo = pers.tile([128, 2, 2, 450], fp8, tag="rhsd_lo")
            rhsd_hi = pers.tile([128, 2, 2, 450], fp8, tag="rhsd_hi")
            rhsb_lo = pers.tile([128, 2, 2, 450], fp8, tag="rhsb_lo")
            rhsb_hi = pers.tile([128, 2, 2, 450], fp8, tag="rhsb_hi")
            lhsT0 = pers.tile([128, 2, 512], fp8, tag="lhsT0")
            lhsTr = pers.tile([128, 3, 2, 512], fp8, tag="lhsTr")
            nc.sync.dma_start(lhsT0[:], d_lhsT[:, 0])
            nc.sync.dma_start(rhsd_lo[:], d_rhsd[:, 0:2])
            nc.sync.dma_start(rhsb_lo[:], d_rhsb[:, 0:2])
            nc.sync.dma_start(rhsd_hi[:], d_rhsd[:, 2:4])
            nc.sync.dma_start(rhsb_hi[:], d_rhsb[:, 2:4])
            nc.sync.dma_start(ident[:], d_ident[:])
            nc.sync.dma_start(lhsTr[:], d_lhsT[:, 1:4])
            rhsd = [rhsd_lo[:, 0], rhsd_lo[:, 1], rhsd_hi[:, 0], rhsd_hi[:, 1]]
            rhsb = [rhsb_lo[:, 0], rhsb_lo[:, 1], rhsb_hi[:, 0], rhsb_hi[:, 1]]

            def lt(t):
                q, m = t // 4, (t % 4) * 128
                if q == 0:
                    return lhsT0[:, :, m:m + 128]
                return lhsTr[:, q - 1, :, m:m + 128]

            state = {"out16": None}
            prev = None  # (M-psum tile, u tile, t, h)
            for step in range(NH):
                t, h = step // 2, step % 2
                if step == 1:
                    # startup: retire step 0 before step 1's DMA-gated mms
                    _finish(nc, outp, d_top8, prev, f16, state)
                pd = psd.tile([128, 2, 512], f32, tag="pd")
                for k in range(2):
                    nc.tensor.matmul(
                        out=pd[:, k, 0:450], lhsT=lt(t),
                        rhs=rhsd[2 * h + k],
                        start=True, stop=True, perf_mode=DR,
                    )
                pm = psm.tile([128, 2, 512], f32, tag="pm")
                for k in range(2):
                    nc.tensor.matmul(
                        out=pm[:, k, 0:450], lhsT=lt(t),
                        rhs=rhsb[2 * h + k],
                        start=True, stop=False, perf_mode=DR,
                    )
                if prev is not None and step != 1:
                    _finish(nc, outp, d_top8, prev, f16, state)
                u = ubp.tile([128, 900], bf16, tag="u")
                nc.scalar.activation(out=u[:], in_=pd[:, :, 0:450], func=Act.Relu)
                prev = (pm, u, ident, t, h)
            _finish(nc, outp, d_top8, prev, f16, state)

    nc.compile()
    return nc


def _finish(nc, outp, d_top8, prev, f16, state):
    pm, u, ident, t, h = prev
    for k in range(2):
        nc.tensor.matmul(
            out=pm[:, k, 0:450], lhsT=ident[:],
            rhs=u[:, k * 450:(k + 1) * 450],
            start=False, stop=True, skip_group_check=True,
        )
    if h == 0:
        state["out16"] = outp.tile([128, 16], f16, tag="out16", name=f"out16_{t}")
    out16 = state["out16"]
    nc.vector.max(out=out16[:, h * 8:(h + 1) * 8], in_=pm[:, :, 0:450])
    ns = slice(t * 128, (t + 1) * 128)
    if t == NTILE - 1:
        # last tile: ship each half as soon as it's ready to shorten the tail
        nc.sync.dma_start(d_top8[ns, h * 8:(h + 1) * 8], out16[:, h * 8:(h + 1) * 8])
    elif h == 1:
        nc.sync.dma_start(d_top8[ns, :], out16[:])


def get_nc():
    if "nc" not in _NC_CACHE:
        _NC_CACHE["nc"] = _build_nc()
    return _NC_CACHE["nc"]


def make_in_maps(w_kp1, kp1_desc, desc2):
    in_maps = []
    ctx = {"w_kp1": np.asarray(w_kp1, np.float32),
           "kq8": [], "dqb8": [], "dqd8": []}
    ident = np.eye(128, dtype=BF16)
    for b in range(B):
        kpd = np.asarray(kp1_desc[b], np.float32)          # N,C
        d2f = np.asarray(desc2[b], np.float32).reshape(C, HW)
        kq8 = (kpd * SCALE).astype(F8)                     # N,C
        dqb8 = (d2f[:, P:] * SCALE).astype(F8)             # C,P
        dqd8 = ((d2f[:, :P] - d2f[:, P:]) * SCALE).astype(F8)
        ctx["kq8"].append(kq8)
        ctx["dqb8"].append(dqb8)
        ctx["dqd8"].append(dqd8)
        # [128, piece, ktile, width] with each piece contiguous per partition
        lhsT = np.ascontiguousarray(
            kq8.T.reshape(2, 128, 4, 512).transpose(1, 2, 0, 3))
        rhsb = np.ascontiguousarray(
            dqb8.reshape(2, 128, 4, 450).transpose(1, 2, 0, 3))
        rhsd = np.ascontiguousarray(
            dqd8.reshape(2, 128, 4, 450).transpose(1, 2, 0, 3))
        in_maps.append({"lhsT": lhsT, "rhsb": rhsb, "rhsd": rhsd, "ident": ident})
    ctx["kp1_desc"] = np.asarray(kp1_desc, np.float32)
    ctx["desc2"] = np.asarray(desc2, np.float32)
    _HOST_CTX.clear()
    _HOST_CTX.update(ctx)
    return in_maps


def finish_loss(results):
    w_kp1 = _HOST_CTX["w_kp1"]
    kp1_desc = _HOST_CTX["kp1_desc"]
    desc2 = _HOST_CTX["desc2"]

    yc = (np.arange(H, dtype=np.float32) + np.float32(0.5)) * np.float32(GRID)
    offs = np.array([(dy, dx) for dy in (-1, 0, 1) for dx in (-1, 0, 1)], np.int32)

    total = 0.0
    for b in range(B):
        wb = w_kp1[b]
        kpd = kp1_desc[b]
        d2f = desc2[b].reshape(C, HW)

        # positive similarity (exact fp32, mirrors reference)
        cy = np.clip(np.floor(wb[:, 0] / np.float32(GRID)).astype(np.int32), 0, H - 1)
        cx = np.clip(np.floor(wb[:, 1] / np.float32(GRID)).astype(np.int32), 0, W - 1)
        fidx = cy * W + cx
        pos = np.einsum("nc,cn->n", kpd, d2f[:, fidx]).astype(np.float32)

        # masked cells: centers within GRID px of the warped keypoint
        h0 = np.clip(np.round((wb[:, 0] - 4.0) / 8.0).astype(np.int32), 0, H - 1)
        w0 = np.clip(np.round((wb[:, 1] - 4.0) / 8.0).astype(np.int32), 0, W - 1)
        hh = h0[:, None] + offs[None, :, 0]
        ww = w0[:, None] + offs[None, :, 1]
        valid = (hh >= 0) & (hh < H) & (ww >= 0) & (ww < W)
        hhc = np.clip(hh, 0, H - 1)
        wwc = np.clip(ww, 0, W - 1)
        d2c = (wb[:, 0:1] - yc[hhc]) ** 2 + (wb[:, 1:2] - yc[wwc]) ** 2
        masked = valid & (d2c <= np.float32(GRID * GRID))
        midx = hhc * W + wwc                                   # N,9

        cand = results[b]["top8"].astype(np.float32)           # N,16
        cand = -np.sort(-cand, axis=1)

        # masked pairs through the same quantized arithmetic
        kq = _HOST_CTX["kq8"][b].astype(np.float32)
        dqb = _HOST_CTX["dqb8"][b].astype(np.float32)
        dqd = _HOST_CTX["dqd8"][b].astype(np.float32)
        pj = np.where(midx < P, midx, midx - P)
        is_a = midx < P
        bq = np.einsum("nc,nkc->nk", kq, dqb[:, pj].transpose(1, 2, 0))
        Dq = np.einsum("nc,nkc->nk", kq, dqd[:, pj].transpose(1, 2, 0))
        uq = np.maximum(Dq, 0.0).astype(BF16).astype(np.float32)
        Mq = (bq + uq).astype(np.float16).astype(np.float32)

        thresh = cand[:, 7] - np.float32(0.5)
        hit = masked & (Mq >= thresh[:, None]) & np.where(is_a, Dq > 0, Dq <= 0)
        any_hit = hit.any(axis=1)

        neg4 = np.empty((N, 4), np.float32)
        neg4[~any_hit] = cand[~any_hit, :4]
        for n in np.nonzero(any_hit)[0]:
            vals = list(cand[n])
            for j in range(9):
                if not hit[n, j]:
                    continue
                m = Mq[n, j]
                eps = max(0.5, abs(m) * 2.0 ** -8)
                bd, best = 1e9, -1
                for i, v in enumerate(vals):
                    d = abs(v - m)
                    if d < bd:
                        bd, best = d, i
                if best >= 0 and bd <= eps:
                    vals.pop(best)
                    ins = bq[n, j] if is_a[n, j] else bq[n, j] + Dq[n, j]
                    vals.append(np.float32(ins))
            vals = sorted(vals, reverse=True)
            neg4[n] = vals[:4]

        neg4 = neg4 / SCALE2
        t = np.maximum(neg4 - pos[:, None] + np.float32(1.0), 0.0)
        total += float((t.astype(np.float64) ** 2).sum())

    return np.asarray(np.float32(total / (B * N * 4)))


def kernel(kp1, w_kp1, kp1_desc, desc2, homo12):
    from concourse.bass_utils import run_bass_kernel_spmd

    nc = get_nc()
    in_maps = make_in_maps(w_kp1, kp1_desc, desc2)
    res = run_bass_kernel_spmd(nc, in_maps, core_ids=list(range(B)))
    return finish_loss(res.results)
